# revision 41
# baseline (speedup 1.0000x reference)
"""Multi-head attention Trainium2 kernel (v2: interleaved PE/ScalarE stream).

Problem: B=4, S=2048, D_MODEL=1024, H=16 heads, d_k=d_v=64.

Sharding (8 cores, no collectives): core c handles batch b=c//2 and head
group g=c%2 (8 heads). Each core computes its 8 heads' attention and the
partial output projection ctx @ Wo[g's rows]; the host sums the two
head-group partials per batch and adds the (folded) biases.

Math notes:
 - bk drops out of softmax exactly; bv and bo fold into a host-side row
   vector bo_eff = bo + bv@Wo; softmax without max subtraction (|score|<~3).
 - matmuls in bf16 with fp32 PSUM accumulation.

Engine budget per core (model): PE 1280 matmul slots ~276us; ScalarE exp
33.5M elems ~255us; DVE ~130us; GPSIMD broadcasts. v1 serialized PE and
ScalarE per head-pair (474us). v2 streams macro-steps (pair p, q-chunk qc):
each step emits 16 score slots interleaved with the PREVIOUS step's ctx
accumulation chains, so the PE always has exp-independent work while
ScalarE chews through exp. ctx uses single K=128 accumulation chains
(was: split halves + DVE merge). ScalarE does exp ONLY; all copies on DVE.
"""

import numpy as np
import ml_dtypes
from collections import deque

import concourse.bass as bass
import concourse.bacc as bacc
import concourse.mybir as mybir
import concourse.tile as tile
from concourse.bass import ts

BF16 = mybir.dt.bfloat16
F32 = mybir.dt.float32

D_MODEL, D_K, D_V, N_HEADS = 1024, 64, 64, 16
B, S = 4, 2048
N_CORES = 8
NH = 8            # heads per core
HD = NH * D_V     # 512, stacked head dim per core
T = S             # tokens per core (one batch)
DC = 8            # D_MODEL / 128 chunks
TCN = 4           # token chunks of 512 for projections
SCN = 16          # s tiles of 128
QCN = 4           # q chunks of 512
HCN = 4           # hd chunks of 128 (2 heads each)
EXP_BUFS = 36
PS_BUFS = 2


def build_nc(reps: int = 1, phases: str = "all"):
    nc = bacc.Bacc("TRN2", target_bir_lowering=False, debug=False)

    xq_d = nc.dram_tensor("xq_t", [128, DC, T], BF16, kind="ExternalInput")
    xk_d = nc.dram_tensor("xk_t", [128, DC, T], BF16, kind="ExternalInput")
    # xv pre-chunked by s-tile on host: [p, sc, dc, t128] -> 4KB DMA lines
    xv_d = nc.dram_tensor("xv_t", [128, SCN, DC, 128], BF16, kind="ExternalInput")
    wq_d = nc.dram_tensor("wq", [128, DC, HD], BF16, kind="ExternalInput")
    wk_d = nc.dram_tensor("wk", [128, DC, HD], BF16, kind="ExternalInput")
    wv_d = nc.dram_tensor("wv", [128, DC, HD], BF16, kind="ExternalInput")
    wo_d = nc.dram_tensor("wo", [128, HCN, D_MODEL], BF16, kind="ExternalInput")
    bq_d = nc.dram_tensor("bq", [128, HCN], F32, kind="ExternalInput")
    out_d = nc.dram_tensor("out", [SCN, 128, D_MODEL], F32, kind="ExternalOutput")

    with tile.TileContext(nc) as tc:
        def body():
            emit_body(nc, tc, xq_d, xk_d, xv_d, wq_d, wk_d, wv_d, wo_d, bq_d, out_d, phases)

        if reps == 1:
            body()
        else:
            with tc.For_i(0, reps, 1):
                body()
    nc.compile()
    return nc


def emit_body(nc, tc, xq_d, xk_d, xv_d, wq_d, wk_d, wv_d, wo_d, bq_d, out_d, phases="all"):
    import contextlib

    ctx = contextlib.ExitStack()
    with ctx:
        # ---------------- persistent SBUF pools ----------------
        wpool = ctx.enter_context(tc.tile_pool(name="wpool", bufs=1))
        qkpool = ctx.enter_context(tc.tile_pool(name="qkpool", bufs=1))
        vpool = ctx.enter_context(tc.tile_pool(name="vpool", bufs=1))
        cpool = ctx.enter_context(tc.tile_pool(name="cpool", bufs=1))
        opool = ctx.enter_context(tc.tile_pool(name="opool", bufs=4))
        zpool = ctx.enter_context(tc.tile_pool(name="zpool", bufs=2))

        wq_sb = wpool.tile([128, DC, HD], BF16, tag="wq")
        wk_sb = wpool.tile([128, DC, HD], BF16, tag="wk")
        wv_sb = wpool.tile([128, DC, HD], BF16, tag="wv")
        wo_sb = wpool.tile([128, HCN, D_MODEL], BF16, tag="wo")
        bq_sb = wpool.tile([128, HCN], F32, tag="bq")

        # weights on the gpsimd DGE ring: parallel with x loads on sync
        nc.gpsimd.dma_start(wq_sb[:], wq_d[:])
        nc.gpsimd.dma_start(bq_sb[:], bq_d[:])
        nc.gpsimd.dma_start(wk_sb[:], wk_d[:])
        nc.gpsimd.dma_start(wv_sb[:], wv_d[:])
        nc.gpsimd.dma_start(wo_sb[:], wo_d[:])

        qhT = qkpool.tile([128, HCN, T], BF16, tag="qhT")  # [hd%128, hd//128, t]
        khT = qkpool.tile([128, HCN, T], BF16, tag="khT")
        vha = vpool.tile([128, SCN, NH, D_V + 1], BF16, tag="vha")  # [s%128, s//128, h, dv|1]
        ctxT = cpool.tile([128, HCN, T], BF16, tag="ctxT")

        nc.vector.memset(vha[:, :, :, D_V : D_V + 1], 1.0)

        if phases == "min":
            # near-empty body: measures per-iteration fixed overhead
            for qt in range(4):
                o_sb = opool.tile([128, 512], F32, tag="o", name="o_min")
                nc.vector.memset(o_sb[:], 0.0)
                nc.sync.dma_start(out_d[qt, :, 0:512], o_sb[:])
            return

        # ---------------- phase A: q/k projections ----------------
        if phases in ("all", "proj", "noexp"):
         with (
            tc.tile_pool(name="xbf", bufs=3) as xbfp,
            tc.tile_pool(name="pp", bufs=6, space="PSUM") as pp,
        ):
            # chunk-interleaved q/k: q chunks load on the sync DGE ring, k
            # chunks on the scalar ring (ScalarE is idle until the first
            # scores land) — the two 8MB streams overlap instead of
            # serializing, and MMs pipeline behind both.
            # Prelude: q fully; k only head-chunks hc0/hc1 (pairs 0-1). khT for
            # hc2/hc3 is produced by a background reload pass inside the
            # stream (k2_units) — shortens the dead head before first scores.
            for tc_i in range(TCN):
                for x_d, w_sb, dst, has_bias, hcs in (
                    (xq_d, wq_sb, qhT, True, range(HCN)),
                    (xk_d, wk_sb, khT, False, range(2)),
                ):
                    xb = xbfp.tile([128, DC, 512], BF16, tag="xb", name="xb_t")
                    (nc.sync if has_bias else nc.scalar).dma_start(
                        xb[:], x_d[:, :, ts(tc_i, 512)]
                    )
                    ptiles = {hc: pp.tile([128, 512], F32, tag="pp", name="pp_t") for hc in hcs}
                    for dc in range(DC):
                        for hc in hcs:
                            nc.tensor.matmul(
                                ptiles[hc][:],
                                lhsT=w_sb[:, dc, ts(hc, 128)],
                                rhs=xb[:, dc, :],
                                start=(dc == 0),
                                stop=(dc == DC - 1),
                            )
                    for hc in hcs:
                        if has_bias:
                            nc.vector.tensor_scalar_add(
                                dst[:, hc, ts(tc_i, 512)], ptiles[hc][:], bq_sb[:, hc : hc + 1]
                            )
                        else:
                            nc.vector.tensor_copy(dst[:, hc, ts(tc_i, 512)], ptiles[hc][:])

        if phases == "proj":
            for qt in range(4):
                o_sb = opool.tile([128, 512], F32, tag="o", name="o_probe")
                nc.vector.tensor_copy(o_sb[:], qhT[:, qt, 0:512])
                nc.sync.dma_start(out_d[qt, :, 0:512], o_sb[:])
            return
        if phases in ("sx", "sonly"):
            nc.vector.memset(qhT[:], 0.01)
            nc.vector.memset(khT[:], 0.01)

        # ---------------- phase B: streamed attention ----------------
        with (
            tc.tile_pool(name="vstageb", bufs=2) as vstageb,
            tc.tile_pool(name="expool", bufs=EXP_BUFS) as expool,
            tc.tile_pool(name="ps", bufs=PS_BUFS, space="PSUM") as ps,
            tc.tile_pool(name="cp", bufs=4, space="PSUM") as cp,
        ):
            exp_tiles = {}
            dummy_e = None
            if phases == "noexp":
                dummy_e = vpool.tile([128, 1024], BF16, tag="dummy_e")
                nc.vector.memset(dummy_e[:], 0.01)

            def scores_slot(p, qc, sc):
                # one PE slot: both heads of pair p via row-tiled K=64 matmuls
                s_ps = ps.tile([128, 1024], F32, tag="ps", name="s_ps")
                for hl in range(2):
                    pb = hl * 64
                    nc.tensor.matmul(
                        s_ps[:, ts(hl, 512)],
                        lhsT=khT[pb : pb + 64, p, ts(sc, 128)],
                        rhs=qhT[pb : pb + 64, p, ts(qc, 512)],
                        start=True,
                        stop=True,
                    )
                if phases == "sonly":
                    return
                if phases == "noexp":
                    exp_tiles[(p, sc, qc)] = dummy_e
                    return
                e = expool.tile([128, 1024], BF16, tag="exp", name="e_t")
                nc.scalar.activation(
                    e[:], s_ps[:], mybir.ActivationFunctionType.Exp, scale=0.125
                )
                exp_tiles[(p, sc, qc)] = e

            def vh_units():
                # vh: psum (128 s-tile, 512 hd) = xT[d, s].T @ W[d], contraction
                # split into d%128 halves 0-63 / 64-127 — the two K=64 chains
                # run concurrently on the row-tiled PE array (full-array K=128
                # matmuls would serialize against the scores/ctx pair stream).
                # DVE merges the two PSUM partials into vha.
                for vc in range(SCN // 2):
                    xb = vstageb.tile([128, 2, DC, 128], BF16, tag="vxb", name="vxb_t")
                    nc.gpsimd.dma_start(xb[:], xv_d[:, 2 * vc : 2 * vc + 2, :, :])
                    yield
                    for si in range(2):
                        sc = 2 * vc + si
                        pv_a = cp.tile([128, 512], F32, tag="cp", name="pv_a")
                        pv_b = cp.tile([128, 512], F32, tag="cp", name="pv_b")
                        for dc in range(DC):
                            nc.tensor.matmul(
                                pv_a[:],
                                lhsT=xb[0:64, si, dc, :],
                                rhs=wv_sb[0:64, dc, :],
                                start=(dc == 0),
                                stop=(dc == DC - 1),
                            )
                            nc.tensor.matmul(
                                pv_b[:],
                                lhsT=xb[64:128, si, dc, :],
                                rhs=wv_sb[64:128, dc, :],
                                start=(dc == 0),
                                stop=(dc == DC - 1),
                            )
                            yield
                        vb_sb = zpool.tile([128, 512], F32, tag="vbs", name="vb_sb")
                        nc.vector.tensor_copy(vb_sb[:], pv_b[:])
                        nc.vector.tensor_add(
                            vha[:, sc, :, 0:D_V],
                            pv_a[:].rearrange("p (h d) -> p h d", d=D_V),
                            vb_sb[:].rearrange("p (h d) -> p h d", d=D_V),
                        )
                        yield

            def ctx_units(p, qc):
                # per head: two K=64 row-tiled accumulation chains (rows 0-63 /
                # 64-127 run concurrently on the PE array); row 64 = Z.
                # Halves merged + normalized on DVE.
                for hl in range(2):
                    h = 2 * p + hl
                    pb = hl * 64
                    c_a = cp.tile([128, 512], F32, tag="cp", name="c_a")
                    c_b = cp.tile([128, 512], F32, tag="cp", name="c_b")
                    for sc in range(SCN):
                        e = exp_tiles[(p, sc, qc)]
                        nc.tensor.matmul(
                            c_a[0 : D_V + 1, :],
                            lhsT=vha[0:64, sc, h, :],
                            rhs=e[0:64, ts(hl, 512)],
                            start=(sc == 0),
                            stop=(sc == SCN - 1),
                        )
                        nc.tensor.matmul(
                            c_b[0 : D_V + 1, :],
                            lhsT=vha[64:128, sc, h, :],
                            rhs=e[64:128, ts(hl, 512)],
                            start=(sc == 0),
                            stop=(sc == SCN - 1),
                        )
                        yield
                    cb_sb = zpool.tile([D_V + 1, 512], F32, tag="cbs", name="cb_sb")
                    nc.vector.tensor_copy(cb_sb[:], c_b[0 : D_V + 1, :])
                    t1 = zpool.tile([D_V + 1, 512], F32, tag="t1", name="t1_t")
                    nc.vector.tensor_add(t1[:], c_a[0 : D_V + 1, :], cb_sb[:])
                    rz = zpool.tile([1, 512], F32, tag="rz", name="rz_t")
                    nc.vector.reciprocal(rz[:], t1[D_V : D_V + 1, :])
                    bc_sb = zpool.tile([64, 512], F32, tag="bcs", name="bc_sb")
                    nc.gpsimd.partition_broadcast(bc_sb[:], rz[:], channels=64)
                    nc.vector.tensor_mul(
                        ctxT[pb : pb + 64, p, ts(qc, 512)], t1[0:D_V, :], bc_sb[:]
                    )
                    yield
                if phases != "noexp":
                    for sc in range(SCN):
                        del exp_tiles[(p, sc, qc)]

            def k2_units():
                # background reload of xk to produce khT for hc2/hc3, also as
                # row-tiled K=64 half-chains + DVE merge (see vh_units).
                for kc in range(DC):
                    xb = vstageb.tile([128, DC, 256], BF16, tag="vxb", name="kxb_t")
                    nc.scalar.dma_start(xb[:], xk_d[:, :, ts(kc, 256)])
                    yield
                    for hc in (2, 3):
                        pt_a = cp.tile([128, 512], F32, tag="cp", name="kp_a")
                        pt_b = cp.tile([128, 512], F32, tag="cp", name="kp_b")
                        for dc in range(DC):
                            nc.tensor.matmul(
                                pt_a[0:128, 0:256],
                                lhsT=wk_sb[0:64, dc, ts(hc, 128)],
                                rhs=xb[0:64, dc, :],
                                start=(dc == 0),
                                stop=(dc == DC - 1),
                            )
                            nc.tensor.matmul(
                                pt_b[0:128, 0:256],
                                lhsT=wk_sb[64:128, dc, ts(hc, 128)],
                                rhs=xb[64:128, dc, :],
                                start=(dc == 0),
                                stop=(dc == DC - 1),
                            )
                            yield
                        kb_sb = zpool.tile([128, 256], F32, tag="vbs", name="kb_sb")
                        nc.vector.tensor_copy(kb_sb[:], pt_b[0:128, 0:256])
                        nc.vector.tensor_add(
                            khT[:, hc, ts(kc, 256)], pt_a[0:128, 0:256], kb_sb[:]
                        )
                        yield

            def out_units(qc):
                for qt in range(4 * qc, 4 * qc + 4):
                    po = [
                        cp.tile([128, 512], F32, tag="cp", name="po_t") for _ in range(2)
                    ]
                    for hc in range(HCN):
                        for d2 in range(2):
                            nc.tensor.matmul(
                                po[d2][:],
                                lhsT=ctxT[:, hc, ts(qt, 128)],
                                rhs=wo_sb[:, hc, ts(d2, 512)],
                                start=(hc == 0),
                                stop=(hc == HCN - 1),
                            )
                        yield
                    for d2 in range(2):
                        o_sb = opool.tile([128, 512], F32, tag="o", name="o_sb")
                        nc.vector.tensor_copy(o_sb[:], po[d2][:])
                        nc.sync.dma_start(out_d[qt, :, ts(d2, 512)], o_sb[:])
                    yield

            # urgent: ctx/out work that must land within its step (e-pool
            # recycling depends on it); bg: vh proj + khT hc2/3 reload.
            urgent = deque()
            bg = deque()

            def drain_from(q, n):
                done = 0
                while done < n and q:
                    try:
                        next(q[0])
                        done += 1
                    except StopIteration:
                        q.popleft()

            def drain_all():
                while urgent:
                    drain_from(urgent, 10**9)
                while bg:
                    drain_from(bg, 10**9)

            if phases not in ("sx", "sonly"):
                bg.append(vh_units())
                bg.append(k2_units())

            # Emission-order invariant (deadlock safety): scores slot allocating
            # exp-buf #N stalls ScalarE until the ctx matmuls consuming buf
            # #(N-EXP_BUFS) run; those must already be EMITTED (earlier in the
            # PE queue). E=32 covers 2 macro-steps of lookahead, so ctx(ms-1)
            # must be fully emitted during step ms — hence vh proj (144 units)
            # must drain entirely within step 0 (budget 9/slot), and ctx+out
            # (<=46 units) within one step (budget 3/slot + 8 boundary).
            # Burst interleaving: keep each PE stretch pattern-homogeneous
            # (scores pairs / ctx pairs / bg pairs) so row-tiled concurrency
            # pipelines instead of restarting at every instruction-mix switch.
            for ms in range(NH // 2 * QCN):
                p, qc = divmod(ms, QCN)
                for burst in range(SCN // 2):
                    scores_slot(p, qc, 2 * burst)
                    scores_slot(p, qc, 2 * burst + 1)
                    if ms == 0:
                        drain_from(bg, 19)
                    else:
                        drain_from(urgent, 6)
                        drain_from(bg, 4)
                drain_from(urgent, 4)
                drain_from(bg, 4)
                if phases == "sonly":
                    continue
                if phases == "sx":
                    for sc in range(SCN):
                        del exp_tiles[(p, sc, qc)]
                    continue
                urgent.append(ctx_units(p, qc))
                if p == NH // 2 - 1:
                    urgent.append(out_units(qc))
            drain_all()

            if phases in ("sx", "sonly"):
                for qt in range(4):
                    o_sb = opool.tile([128, 512], F32, tag="o", name="o_probe2")
                    nc.vector.tensor_copy(o_sb[:], qhT[:, qt, 0:512])
                    nc.sync.dma_start(out_d[qt, :, 0:512], o_sb[:])


# ---------------------------------------------------------------------------
# host side
# ---------------------------------------------------------------------------

_NC_CACHE = {}


def _get_nc(reps: int = 1):
    if reps not in _NC_CACHE:
        _NC_CACHE[reps] = build_nc(reps)
    return _NC_CACHE[reps]


def _to_bf16(a):
    return np.ascontiguousarray(a).astype(ml_dtypes.bfloat16)


def make_in_maps(q, k, v, Wq, bq, Wk, bk, Wv, bv, Wo, bo):
    """Build the per-core input maps (host-side sharding + layout)."""
    in_maps = []
    for c in range(N_CORES):
        b = c // 2
        hg = c % 2
        hs = slice(hg * NH, hg * NH + NH)

        def xt(x):
            # (S, D) -> [p, dc, t] with D = dc*128 + p, shipped as bf16
            return _to_bf16(
                np.asarray(x, np.float32).T.reshape(DC, 128, T).transpose(1, 0, 2)
            )

        def xt_v(x):
            # (S, D) -> [p, sc, dc, t128]: s-tile chunked, bf16
            return _to_bf16(
                np.asarray(x, np.float32).T.reshape(DC, 128, SCN, 128).transpose(1, 2, 0, 3)
            )

        def wproj(W):
            # (8, 1024, 64) -> [p, dc, hd]  (hd = h*64+dv, D = dc*128+p)
            Wc = np.asarray(W[hs], np.float32).transpose(1, 0, 2).reshape(D_MODEL, HD)
            return _to_bf16(Wc.reshape(DC, 128, HD).transpose(1, 0, 2))

        wo_c = np.asarray(Wo[hg * HD : (hg + 1) * HD], np.float32)  # (512, 1024)
        bq_c = np.asarray(bq[hs], np.float32).reshape(HD)  # (512,)

        in_maps.append(
            {
                "xq_t": xt(q[b]),
                "xk_t": xt(k[b]),
                "xv_t": xt_v(v[b]),
                "wq": wproj(Wq),
                "wk": wproj(Wk),
                "wv": wproj(Wv),
                "wo": _to_bf16(wo_c.reshape(HCN, 128, D_MODEL).transpose(1, 0, 2)),
                "bq": np.ascontiguousarray(bq_c.reshape(HCN, 128).T),
            }
        )
    return in_maps


def combine_outputs(results, bv, Wo, bo):
    """results: list of 8 dicts with 'out' (16,128,1024). Returns (B,S,D)."""
    bo_eff = np.asarray(bo, np.float32) + np.asarray(bv, np.float32).reshape(-1) @ np.asarray(
        Wo, np.float32
    )
    out = np.empty((B, S, D_MODEL), np.float32)
    for b in range(B):
        p0 = results[2 * b]["out"].reshape(S, D_MODEL)
        p1 = results[2 * b + 1]["out"].reshape(S, D_MODEL)
        out[b] = p0 + p1 + bo_eff
    return out


def kernel(q, k, v, Wq, bq, Wk, bk, Wv, bv, Wo, bo):
    from concourse.bass_utils import run_bass_kernel_spmd

    nc = _get_nc(1)
    in_maps = make_in_maps(q, k, v, Wq, bq, Wk, bk, Wv, bv, Wo, bo)
    res = run_bass_kernel_spmd(nc, in_maps, core_ids=list(range(N_CORES)))
    return combine_outputs(res.results, bv, Wo, bo)


# revision 48
# speedup vs baseline: 1.0015x; 1.0015x over previous
"""Multi-head attention Trainium2 kernel (v2: interleaved PE/ScalarE stream).

Problem: B=4, S=2048, D_MODEL=1024, H=16 heads, d_k=d_v=64.

Sharding (8 cores, no collectives): core c handles batch b=c//2 and head
group g=c%2 (8 heads). Each core computes its 8 heads' attention and the
partial output projection ctx @ Wo[g's rows]; the host sums the two
head-group partials per batch and adds the (folded) biases.

Math notes:
 - bk drops out of softmax exactly; bv and bo fold into a host-side row
   vector bo_eff = bo + bv@Wo; softmax without max subtraction (|score|<~3).
 - matmuls in bf16 with fp32 PSUM accumulation.

Engine budget per core (model): PE 1280 matmul slots ~276us; ScalarE exp
33.5M elems ~255us; DVE ~130us; GPSIMD broadcasts. v1 serialized PE and
ScalarE per head-pair (474us). v2 streams macro-steps (pair p, q-chunk qc):
each step emits 16 score slots interleaved with the PREVIOUS step's ctx
accumulation chains, so the PE always has exp-independent work while
ScalarE chews through exp. ctx uses single K=128 accumulation chains
(was: split halves + DVE merge). ScalarE does exp ONLY; all copies on DVE.
"""

import numpy as np
import ml_dtypes
from collections import deque

import concourse.bass as bass
import concourse.bacc as bacc
import concourse.mybir as mybir
import concourse.tile as tile
from concourse.bass import ts

BF16 = mybir.dt.bfloat16
F32 = mybir.dt.float32

D_MODEL, D_K, D_V, N_HEADS = 1024, 64, 64, 16
B, S = 4, 2048
N_CORES = 8
NH = 8            # heads per core
HD = NH * D_V     # 512, stacked head dim per core
T = S             # tokens per core (one batch)
DC = 8            # D_MODEL / 128 chunks
TCN = 4           # token chunks of 512 for projections
SCN = 16          # s tiles of 128
QCN = 4           # q chunks of 512
HCN = 4           # hd chunks of 128 (2 heads each)
EXP_BUFS = 36
PS_BUFS = 2


def build_nc(reps: int = 1, phases: str = "all"):
    nc = bacc.Bacc("TRN2", target_bir_lowering=False, debug=False)

    xq_d = nc.dram_tensor("xq_t", [128, DC, T], BF16, kind="ExternalInput")
    xk_d = nc.dram_tensor("xk_t", [128, DC, T], BF16, kind="ExternalInput")
    # xv pre-chunked by s-tile on host: [p, sc, dc, t128] -> 4KB DMA lines
    xv_d = nc.dram_tensor("xv_t", [128, SCN, DC, 128], BF16, kind="ExternalInput")
    wq_d = nc.dram_tensor("wq", [128, DC, HD], BF16, kind="ExternalInput")
    wk_d = nc.dram_tensor("wk", [128, DC, HD], BF16, kind="ExternalInput")
    wv_d = nc.dram_tensor("wv", [128, DC, HD], BF16, kind="ExternalInput")
    wo_d = nc.dram_tensor("wo", [128, HCN, D_MODEL], BF16, kind="ExternalInput")
    bq_d = nc.dram_tensor("bq", [128, HCN], F32, kind="ExternalInput")
    out_d = nc.dram_tensor("out", [SCN, 128, D_MODEL], F32, kind="ExternalOutput")

    with tile.TileContext(nc) as tc:
        def body():
            emit_body(nc, tc, xq_d, xk_d, xv_d, wq_d, wk_d, wv_d, wo_d, bq_d, out_d, phases)

        if reps == 1:
            body()
        else:
            with tc.For_i(0, reps, 1):
                body()
    nc.compile()
    return nc


def emit_body(nc, tc, xq_d, xk_d, xv_d, wq_d, wk_d, wv_d, wo_d, bq_d, out_d, phases="all"):
    import contextlib

    ctx = contextlib.ExitStack()
    with ctx:
        # ---------------- persistent SBUF pools ----------------
        wpool = ctx.enter_context(tc.tile_pool(name="wpool", bufs=1))
        qkpool = ctx.enter_context(tc.tile_pool(name="qkpool", bufs=1))
        vpool = ctx.enter_context(tc.tile_pool(name="vpool", bufs=1))
        cpool = ctx.enter_context(tc.tile_pool(name="cpool", bufs=1))
        opool = ctx.enter_context(tc.tile_pool(name="opool", bufs=4))
        zpool = ctx.enter_context(tc.tile_pool(name="zpool", bufs=2))

        wq_sb = wpool.tile([128, DC, HD], BF16, tag="wq")
        wk_sb = wpool.tile([128, DC, HD], BF16, tag="wk")
        wv_sb = wpool.tile([128, DC, HD], BF16, tag="wv")
        wo_sb = wpool.tile([128, HCN, D_MODEL], BF16, tag="wo")
        bq_sb = wpool.tile([128, HCN], F32, tag="bq")

        # weights on the gpsimd DGE ring: parallel with x loads on sync
        nc.gpsimd.dma_start(wq_sb[:], wq_d[:])
        nc.gpsimd.dma_start(bq_sb[:], bq_d[:])
        nc.gpsimd.dma_start(wk_sb[:], wk_d[:])
        nc.gpsimd.dma_start(wv_sb[:], wv_d[:])
        nc.gpsimd.dma_start(wo_sb[:], wo_d[:])

        qhT = qkpool.tile([128, HCN, T], BF16, tag="qhT")  # [hd%128, hd//128, t]
        khT = qkpool.tile([128, HCN, T], BF16, tag="khT")
        vha = vpool.tile([128, SCN, NH, D_V + 1], BF16, tag="vha")  # [s%128, s//128, h, dv|1]
        ctxT = cpool.tile([128, HCN, T], BF16, tag="ctxT")

        nc.vector.memset(vha[:, :, :, D_V : D_V + 1], 1.0)

        if phases == "min":
            # near-empty body: measures per-iteration fixed overhead
            for qt in range(4):
                o_sb = opool.tile([128, 512], F32, tag="o", name="o_min")
                nc.vector.memset(o_sb[:], 0.0)
                nc.sync.dma_start(out_d[qt, :, 0:512], o_sb[:])
            return

        if phases == "dmaonly":
            # all input DMA traffic, no compute
            nc.gpsimd.dma_start(wq_sb[:], wq_d[:])
            nc.gpsimd.dma_start(wk_sb[:], wk_d[:])
            nc.gpsimd.dma_start(wv_sb[:], wv_d[:])
            nc.gpsimd.dma_start(wo_sb[:], wo_d[:])
            with tc.tile_pool(name="dstage", bufs=3) as dstage:
                for tc_i in range(TCN):
                    xbq = dstage.tile([128, DC, 512], BF16, tag="dq", name="dq_t")
                    nc.sync.dma_start(xbq[:], xq_d[:, :, ts(tc_i, 512)])
                    xbk = dstage.tile([128, DC, 512], BF16, tag="dk", name="dk_t")
                    nc.scalar.dma_start(xbk[:], xk_d[:, :, ts(tc_i, 512)])
                for vc in range(SCN // 2):
                    xbv = dstage.tile([128, 2, DC, 128], BF16, tag="dv", name="dv_t")
                    nc.gpsimd.dma_start(xbv[:], xv_d[:, 2 * vc : 2 * vc + 2, :, :])
            for qt in range(4):
                o_sb = opool.tile([128, 512], F32, tag="o", name="o_dma")
                nc.vector.memset(o_sb[:], 0.0)
                nc.sync.dma_start(out_d[qt, :, 0:512], o_sb[:])
            return

        # ---------------- phase A: q/k projections ----------------
        if phases in ("all", "proj", "noexp"):
         with (
            tc.tile_pool(name="xbf", bufs=3) as xbfp,
            tc.tile_pool(name="pp", bufs=6, space="PSUM") as pp,
        ):
            # chunk-interleaved q/k: q chunks load on the sync DGE ring, k
            # chunks on the scalar ring (ScalarE is idle until the first
            # scores land) — the two 8MB streams overlap instead of
            # serializing, and MMs pipeline behind both.
            # Prelude: q fully; k only head-chunks hc0/hc1 (pairs 0-1). khT for
            # hc2/hc3 is produced by a background reload pass inside the
            # stream (k2_units) — shortens the dead head before first scores.
            for tc_i in range(TCN):
                for x_d, w_sb, dst, has_bias, hcs in (
                    (xq_d, wq_sb, qhT, True, range(HCN)),
                    (xk_d, wk_sb, khT, False, range(2)),
                ):
                    xb = xbfp.tile([128, DC, 512], BF16, tag="xb", name="xb_t")
                    (nc.sync if has_bias else nc.scalar).dma_start(
                        xb[:], x_d[:, :, ts(tc_i, 512)]
                    )
                    ptiles = {hc: pp.tile([128, 512], F32, tag="pp", name="pp_t") for hc in hcs}
                    for dc in range(DC):
                        for hc in hcs:
                            nc.tensor.matmul(
                                ptiles[hc][:],
                                lhsT=w_sb[:, dc, ts(hc, 128)],
                                rhs=xb[:, dc, :],
                                start=(dc == 0),
                                stop=(dc == DC - 1),
                            )
                    for hc in hcs:
                        if has_bias:
                            nc.vector.tensor_scalar_add(
                                dst[:, hc, ts(tc_i, 512)], ptiles[hc][:], bq_sb[:, hc : hc + 1]
                            )
                        else:
                            nc.vector.tensor_copy(dst[:, hc, ts(tc_i, 512)], ptiles[hc][:])

        if phases == "proj":
            for qt in range(4):
                o_sb = opool.tile([128, 512], F32, tag="o", name="o_probe")
                nc.vector.tensor_copy(o_sb[:], qhT[:, qt, 0:512])
                nc.sync.dma_start(out_d[qt, :, 0:512], o_sb[:])
            return
        if phases in ("sx", "sonly", "sc"):
            nc.vector.memset(qhT[:], 0.01)
            nc.vector.memset(khT[:], 0.01)
            if phases == "sc":
                nc.vector.memset(vha[:], 0.01)
                nc.vector.memset(vha[:, :, :, D_V : D_V + 1], 1.0)

        # ---------------- phase B: streamed attention ----------------
        with (
            tc.tile_pool(name="vstageb", bufs=2) as vstageb,
            tc.tile_pool(name="expool", bufs=EXP_BUFS) as expool,
            tc.tile_pool(name="ps", bufs=PS_BUFS, space="PSUM") as ps,
            tc.tile_pool(name="cp", bufs=4, space="PSUM") as cp,
        ):
            exp_tiles = {}
            dummy_e = None
            if phases in ("noexp", "sc"):
                dummy_e = vpool.tile([128, 1024], BF16, tag="dummy_e")
                nc.vector.memset(dummy_e[:], 0.01)

            def scores_slot(p, qc, sc):
                # one PE slot: both heads of pair p via row-tiled K=64 matmuls
                s_ps = ps.tile([128, 1024], F32, tag="ps", name="s_ps")
                for hl in range(2):
                    pb = hl * 64
                    nc.tensor.matmul(
                        s_ps[:, ts(hl, 512)],
                        lhsT=khT[pb : pb + 64, p, ts(sc, 128)],
                        rhs=qhT[pb : pb + 64, p, ts(qc, 512)],
                        start=True,
                        stop=True,
                    )
                if phases == "sonly":
                    return
                if phases in ("noexp", "sc"):
                    exp_tiles[(p, sc, qc)] = dummy_e
                    return
                e = expool.tile([128, 1024], BF16, tag="exp", name="e_t")
                nc.scalar.activation(
                    e[:], s_ps[:], mybir.ActivationFunctionType.Exp, scale=0.125
                )
                exp_tiles[(p, sc, qc)] = e

            def vh_units():
                # vh: psum (128 s-tile, 512 hd) = xT[d, s].T @ W[d], contraction
                # split into d%128 halves 0-63 / 64-127 — the two K=64 chains
                # run concurrently on the row-tiled PE array (full-array K=128
                # matmuls would serialize against the scores/ctx pair stream).
                # DVE merges the two PSUM partials into vha.
                for vc in range(SCN // 2):
                    xb = vstageb.tile([128, 2, DC, 128], BF16, tag="vxb", name="vxb_t")
                    nc.gpsimd.dma_start(xb[:], xv_d[:, 2 * vc : 2 * vc + 2, :, :])
                    yield
                    for si in range(2):
                        sc = 2 * vc + si
                        pv = cp.tile([128, 512], F32, tag="cp", name="pv_t")
                        for dc in range(DC):
                            nc.tensor.matmul(
                                pv[:],
                                lhsT=xb[:, si, dc, :],
                                rhs=wv_sb[:, dc, :],
                                start=(dc == 0),
                                stop=(dc == DC - 1),
                            )
                            yield
                        nc.vector.tensor_copy(
                            vha[:, sc, :, 0:D_V],
                            pv[:].rearrange("p (h d) -> p h d", d=D_V),
                        )
                        yield

            def ctx_units(p, qc):
                # per head: one K=128 accumulation chain over 16 s-tiles; row
                # 64 = Z (accumulation chains don't row-tile-overlap on HW, so
                # a single full-K chain halves the instruction count).
                for hl in range(2):
                    h = 2 * p + hl
                    pb = hl * 64
                    c = cp.tile([128, 512], F32, tag="cp", name="c_t")
                    for sc in range(SCN):
                        e = exp_tiles[(p, sc, qc)]
                        nc.tensor.matmul(
                            c[0 : D_V + 1, :],
                            lhsT=vha[:, sc, h, :],
                            rhs=e[:, ts(hl, 512)],
                            start=(sc == 0),
                            stop=(sc == SCN - 1),
                        )
                        yield
                    rz = zpool.tile([1, 512], F32, tag="rz", name="rz_t")
                    nc.vector.reciprocal(rz[:], c[D_V : D_V + 1, :])
                    bc_sb = zpool.tile([64, 512], F32, tag="bcs", name="bc_sb")
                    nc.gpsimd.partition_broadcast(bc_sb[:], rz[:], channels=64)
                    nc.vector.tensor_mul(
                        ctxT[pb : pb + 64, p, ts(qc, 512)], c[0:D_V, :], bc_sb[:]
                    )
                    yield
                if phases not in ("noexp", "sc"):
                    for sc in range(SCN):
                        del exp_tiles[(p, sc, qc)]

            def k2_units():
                # background reload of xk to produce khT for hc2/hc3, also as
                # row-tiled K=64 half-chains + DVE merge (see vh_units).
                for kc in range(DC):
                    xb = vstageb.tile([128, DC, 256], BF16, tag="vxb", name="kxb_t")
                    nc.scalar.dma_start(xb[:], xk_d[:, :, ts(kc, 256)])
                    yield
                    for hc in (2, 3):
                        pt = cp.tile([128, 512], F32, tag="cp", name="kp_t")
                        for dc in range(DC):
                            nc.tensor.matmul(
                                pt[0:128, 0:256],
                                lhsT=wk_sb[:, dc, ts(hc, 128)],
                                rhs=xb[:, dc, :],
                                start=(dc == 0),
                                stop=(dc == DC - 1),
                            )
                            yield
                        nc.vector.tensor_copy(khT[:, hc, ts(kc, 256)], pt[0:128, 0:256])
                        yield

            def out_units(qc):
                for qt in range(4 * qc, 4 * qc + 4):
                    po = [
                        cp.tile([128, 512], F32, tag="cp", name="po_t") for _ in range(2)
                    ]
                    for hc in range(HCN):
                        for d2 in range(2):
                            nc.tensor.matmul(
                                po[d2][:],
                                lhsT=ctxT[:, hc, ts(qt, 128)],
                                rhs=wo_sb[:, hc, ts(d2, 512)],
                                start=(hc == 0),
                                stop=(hc == HCN - 1),
                            )
                        yield
                    for d2 in range(2):
                        o_sb = opool.tile([128, 512], F32, tag="o", name="o_sb")
                        nc.vector.tensor_copy(o_sb[:], po[d2][:])
                        nc.sync.dma_start(out_d[qt, :, ts(d2, 512)], o_sb[:])
                    yield

            # urgent: ctx/out work that must land within its step (e-pool
            # recycling depends on it); bg: vh proj + khT hc2/3 reload.
            urgent = deque()
            bg = deque()

            def drain_from(q, n):
                done = 0
                while done < n and q:
                    try:
                        next(q[0])
                        done += 1
                    except StopIteration:
                        q.popleft()

            def drain_all():
                while urgent:
                    drain_from(urgent, 10**9)
                while bg:
                    drain_from(bg, 10**9)

            if phases not in ("sx", "sonly", "sc"):
                bg.append(vh_units())
                bg.append(k2_units())

            # Emission-order invariant (deadlock safety): scores slot allocating
            # exp-buf #N stalls ScalarE until the ctx matmuls consuming buf
            # #(N-EXP_BUFS) run; those must already be EMITTED (earlier in the
            # PE queue). E=32 covers 2 macro-steps of lookahead, so ctx(ms-1)
            # must be fully emitted during step ms — hence vh proj (144 units)
            # must drain entirely within step 0 (budget 9/slot), and ctx+out
            # (<=46 units) within one step (budget 3/slot + 8 boundary).
            # Burst interleaving: keep each PE stretch pattern-homogeneous
            # (scores pairs / ctx pairs / bg pairs) so row-tiled concurrency
            # pipelines instead of restarting at every instruction-mix switch.
            for ms in range(NH // 2 * QCN):
                p, qc = divmod(ms, QCN)
                for burst in range(SCN // 2):
                    scores_slot(p, qc, 2 * burst)
                    scores_slot(p, qc, 2 * burst + 1)
                    if ms == 0:
                        drain_from(bg, 19)
                    else:
                        drain_from(urgent, 6)
                        drain_from(bg, 4)
                drain_from(urgent, 4)
                drain_from(bg, 4)
                if phases == "sonly":
                    continue
                if phases == "sx":
                    for sc in range(SCN):
                        del exp_tiles[(p, sc, qc)]
                    continue
                urgent.append(ctx_units(p, qc))
                if p == NH // 2 - 1 and phases != "sc":
                    urgent.append(out_units(qc))
            drain_all()

            if phases in ("sx", "sonly", "sc"):
                for qt in range(4):
                    o_sb = opool.tile([128, 512], F32, tag="o", name="o_probe2")
                    nc.vector.tensor_copy(o_sb[:], qhT[:, qt, 0:512])
                    nc.sync.dma_start(out_d[qt, :, 0:512], o_sb[:])


# ---------------------------------------------------------------------------
# host side
# ---------------------------------------------------------------------------

_NC_CACHE = {}


def _get_nc(reps: int = 1):
    if reps not in _NC_CACHE:
        _NC_CACHE[reps] = build_nc(reps)
    return _NC_CACHE[reps]


def _to_bf16(a):
    return np.ascontiguousarray(a).astype(ml_dtypes.bfloat16)


def make_in_maps(q, k, v, Wq, bq, Wk, bk, Wv, bv, Wo, bo):
    """Build the per-core input maps (host-side sharding + layout)."""
    in_maps = []
    for c in range(N_CORES):
        b = c // 2
        hg = c % 2
        hs = slice(hg * NH, hg * NH + NH)

        def xt(x):
            # (S, D) -> [p, dc, t] with D = dc*128 + p, shipped as bf16
            return _to_bf16(
                np.asarray(x, np.float32).T.reshape(DC, 128, T).transpose(1, 0, 2)
            )

        def xt_v(x):
            # (S, D) -> [p, sc, dc, t128]: s-tile chunked, bf16
            return _to_bf16(
                np.asarray(x, np.float32).T.reshape(DC, 128, SCN, 128).transpose(1, 2, 0, 3)
            )

        def wproj(W):
            # (8, 1024, 64) -> [p, dc, hd]  (hd = h*64+dv, D = dc*128+p)
            Wc = np.asarray(W[hs], np.float32).transpose(1, 0, 2).reshape(D_MODEL, HD)
            return _to_bf16(Wc.reshape(DC, 128, HD).transpose(1, 0, 2))

        wo_c = np.asarray(Wo[hg * HD : (hg + 1) * HD], np.float32)  # (512, 1024)
        bq_c = np.asarray(bq[hs], np.float32).reshape(HD)  # (512,)

        in_maps.append(
            {
                "xq_t": xt(q[b]),
                "xk_t": xt(k[b]),
                "xv_t": xt_v(v[b]),
                "wq": wproj(Wq),
                "wk": wproj(Wk),
                "wv": wproj(Wv),
                "wo": _to_bf16(wo_c.reshape(HCN, 128, D_MODEL).transpose(1, 0, 2)),
                "bq": np.ascontiguousarray(bq_c.reshape(HCN, 128).T),
            }
        )
    return in_maps


def combine_outputs(results, bv, Wo, bo):
    """results: list of 8 dicts with 'out' (16,128,1024). Returns (B,S,D)."""
    bo_eff = np.asarray(bo, np.float32) + np.asarray(bv, np.float32).reshape(-1) @ np.asarray(
        Wo, np.float32
    )
    out = np.empty((B, S, D_MODEL), np.float32)
    for b in range(B):
        p0 = results[2 * b]["out"].reshape(S, D_MODEL)
        p1 = results[2 * b + 1]["out"].reshape(S, D_MODEL)
        out[b] = p0 + p1 + bo_eff
    return out


def kernel(q, k, v, Wq, bq, Wk, bk, Wv, bv, Wo, bo):
    from concourse.bass_utils import run_bass_kernel_spmd

    nc = _get_nc(1)
    in_maps = make_in_maps(q, k, v, Wq, bq, Wk, bk, Wv, bv, Wo, bo)
    res = run_bass_kernel_spmd(nc, in_maps, core_ids=list(range(N_CORES)))
    return combine_outputs(res.results, bv, Wo, bo)


# revision 49
# speedup vs baseline: 1.0138x; 1.0123x over previous
"""Multi-head attention Trainium2 kernel (v2: interleaved PE/ScalarE stream).

Problem: B=4, S=2048, D_MODEL=1024, H=16 heads, d_k=d_v=64.

Sharding (8 cores, no collectives): core c handles batch b=c//2 and head
group g=c%2 (8 heads). Each core computes its 8 heads' attention and the
partial output projection ctx @ Wo[g's rows]; the host sums the two
head-group partials per batch and adds the (folded) biases.

Math notes:
 - bk drops out of softmax exactly; bv and bo fold into a host-side row
   vector bo_eff = bo + bv@Wo; softmax without max subtraction (|score|<~3).
 - matmuls in bf16 with fp32 PSUM accumulation.

Structure (measured-driven):
 - x shipped bf16 from host (halves HBM traffic; device matmuls were bf16
   anyway, bit-identical). xv pre-chunked per s-tile for contiguous DMA.
 - q/k/v loads spread across the sync/scalar/gpsimd DGE rings.
 - Prelude: q proj fully + k proj pairs 0-1; khT for pairs 2-3 comes from a
   background xk reload inside the stream (shorter dead head before the
   first scores/exp).
 - Stream of macro-steps (pair p, q-chunk qc): 2 scores slots (row-tiled
   K=64 pairs DO run concurrently for single-shot matmuls), then bursts of
   the previous step's ctx chains (urgent) and vh-proj/khT-reload units
   (background). ScalarE does exp ONLY (33.5M elems ~ 266us, the pacing
   engine); all PSUM->SBUF copies are on DVE; 1/Z partition-broadcast on
   GPSIMD.
 - ctx/vh/k2/out use single K=128 accumulation chains: row-tiled K=64
   accumulation-chain pairs do NOT overlap on HW (measured ~190ns/MM vs
   ~98ns/MM for single-shot pairs), so full-K chains halve the instruction
   count for the same streaming time. Z comes free as ctx row 64 (ones
   column appended to vh).
"""

import numpy as np
import ml_dtypes
from collections import deque

import concourse.bass as bass
import concourse.bacc as bacc
import concourse.mybir as mybir
import concourse.tile as tile
from concourse.bass import ts

BF16 = mybir.dt.bfloat16
F32 = mybir.dt.float32

D_MODEL, D_K, D_V, N_HEADS = 1024, 64, 64, 16
B, S = 4, 2048
N_CORES = 8
NH = 8            # heads per core
HD = NH * D_V     # 512, stacked head dim per core
T = S             # tokens per core (one batch)
DC = 8            # D_MODEL / 128 chunks
TCN = 4           # token chunks of 512 for projections
SCN = 16          # s tiles of 128
QCN = 4           # q chunks of 512
HCN = 4           # hd chunks of 128 (2 heads each)
EXP_BUFS = 36
PS_BUFS = 2


def build_nc(reps: int = 1, phases: str = "all"):
    nc = bacc.Bacc("TRN2", target_bir_lowering=False, debug=False)

    xq_d = nc.dram_tensor("xq_t", [128, DC, T], BF16, kind="ExternalInput")
    xk_d = nc.dram_tensor("xk_t", [128, DC, T], BF16, kind="ExternalInput")
    # xv pre-chunked by s-tile on host: [p, sc, dc, t128] -> 4KB DMA lines
    xv_d = nc.dram_tensor("xv_t", [128, SCN, DC, 128], BF16, kind="ExternalInput")
    wq_d = nc.dram_tensor("wq", [128, DC, HD], BF16, kind="ExternalInput")
    wk_d = nc.dram_tensor("wk", [128, DC, HD], BF16, kind="ExternalInput")
    wv_d = nc.dram_tensor("wv", [128, DC, HD], BF16, kind="ExternalInput")
    wo_d = nc.dram_tensor("wo", [128, HCN, D_MODEL], BF16, kind="ExternalInput")
    bq_d = nc.dram_tensor("bq", [128, HCN], F32, kind="ExternalInput")
    out_d = nc.dram_tensor("out", [SCN, 128, D_MODEL], F32, kind="ExternalOutput")

    with tile.TileContext(nc) as tc:
        def body():
            emit_body(nc, tc, xq_d, xk_d, xv_d, wq_d, wk_d, wv_d, wo_d, bq_d, out_d, phases)

        if reps == 1:
            body()
        else:
            with tc.For_i(0, reps, 1):
                body()
    nc.compile()
    return nc


def emit_body(nc, tc, xq_d, xk_d, xv_d, wq_d, wk_d, wv_d, wo_d, bq_d, out_d, phases="all"):
    import contextlib

    ctx = contextlib.ExitStack()
    with ctx:
        # ---------------- persistent SBUF pools ----------------
        wpool = ctx.enter_context(tc.tile_pool(name="wpool", bufs=1))
        qkpool = ctx.enter_context(tc.tile_pool(name="qkpool", bufs=1))
        vpool = ctx.enter_context(tc.tile_pool(name="vpool", bufs=1))
        cpool = ctx.enter_context(tc.tile_pool(name="cpool", bufs=1))
        opool = ctx.enter_context(tc.tile_pool(name="opool", bufs=4))
        zpool = ctx.enter_context(tc.tile_pool(name="zpool", bufs=2))

        wq_sb = wpool.tile([128, DC, HD], BF16, tag="wq")
        wk_sb = wpool.tile([128, DC, HD], BF16, tag="wk")
        wv_sb = wpool.tile([128, DC, HD], BF16, tag="wv")
        wo_sb = wpool.tile([128, HCN, D_MODEL], BF16, tag="wo")
        bq_sb = wpool.tile([128, HCN], F32, tag="bq")

        # weights on the gpsimd DGE ring: parallel with x loads on sync
        nc.gpsimd.dma_start(wq_sb[:], wq_d[:])
        nc.gpsimd.dma_start(bq_sb[:], bq_d[:])
        nc.gpsimd.dma_start(wk_sb[:], wk_d[:])
        nc.gpsimd.dma_start(wv_sb[:], wv_d[:])
        nc.gpsimd.dma_start(wo_sb[:], wo_d[:])

        qhT = qkpool.tile([128, HCN, T], BF16, tag="qhT")  # [hd%128, hd//128, t]
        khT = qkpool.tile([128, HCN, T], BF16, tag="khT")
        vha = vpool.tile([128, SCN, NH, D_V + 1], BF16, tag="vha")  # [s%128, s//128, h, dv|1]
        ctxT = cpool.tile([128, HCN, T], BF16, tag="ctxT")

        nc.vector.memset(vha[:, :, :, D_V : D_V + 1], 1.0)

        if phases == "min":
            # near-empty body: measures per-iteration fixed overhead
            for qt in range(4):
                o_sb = opool.tile([128, 512], F32, tag="o", name="o_min")
                nc.vector.memset(o_sb[:], 0.0)
                nc.sync.dma_start(out_d[qt, :, 0:512], o_sb[:])
            return

        if phases == "dmaonly":
            # all input DMA traffic, no compute
            nc.gpsimd.dma_start(wq_sb[:], wq_d[:])
            nc.gpsimd.dma_start(wk_sb[:], wk_d[:])
            nc.gpsimd.dma_start(wv_sb[:], wv_d[:])
            nc.gpsimd.dma_start(wo_sb[:], wo_d[:])
            with tc.tile_pool(name="dstage", bufs=3) as dstage:
                for tc_i in range(TCN):
                    xbq = dstage.tile([128, DC, 512], BF16, tag="dq", name="dq_t")
                    nc.sync.dma_start(xbq[:], xq_d[:, :, ts(tc_i, 512)])
                    xbk = dstage.tile([128, DC, 512], BF16, tag="dk", name="dk_t")
                    nc.scalar.dma_start(xbk[:], xk_d[:, :, ts(tc_i, 512)])
                for vc in range(SCN // 2):
                    xbv = dstage.tile([128, 2, DC, 128], BF16, tag="dv", name="dv_t")
                    nc.gpsimd.dma_start(xbv[:], xv_d[:, 2 * vc : 2 * vc + 2, :, :])
            for qt in range(4):
                o_sb = opool.tile([128, 512], F32, tag="o", name="o_dma")
                nc.vector.memset(o_sb[:], 0.0)
                nc.sync.dma_start(out_d[qt, :, 0:512], o_sb[:])
            return

        # ---------------- phase A: q/k projections ----------------
        if phases in ("all", "proj", "noexp"):
         with (
            tc.tile_pool(name="xbf", bufs=3) as xbfp,
            tc.tile_pool(name="pp", bufs=6, space="PSUM") as pp,
        ):
            # chunk-interleaved q/k: q chunks load on the sync DGE ring, k
            # chunks on the scalar ring (ScalarE is idle until the first
            # scores land) — the two 8MB streams overlap instead of
            # serializing, and MMs pipeline behind both.
            # Prelude: q fully; k only head-chunks hc0/hc1 (pairs 0-1). khT for
            # hc2/hc3 is produced by a background reload pass inside the
            # stream (k2_units) — shortens the dead head before first scores.
            for tc_i in range(TCN):
                for x_d, w_sb, dst, has_bias, hcs in (
                    (xq_d, wq_sb, qhT, True, range(HCN)),
                    (xk_d, wk_sb, khT, False, range(2)),
                ):
                    xb = xbfp.tile([128, DC, 512], BF16, tag="xb", name="xb_t")
                    (nc.sync if has_bias else nc.scalar).dma_start(
                        xb[:], x_d[:, :, ts(tc_i, 512)]
                    )
                    ptiles = {hc: pp.tile([128, 512], F32, tag="pp", name="pp_t") for hc in hcs}
                    for dc in range(DC):
                        for hc in hcs:
                            nc.tensor.matmul(
                                ptiles[hc][:],
                                lhsT=w_sb[:, dc, ts(hc, 128)],
                                rhs=xb[:, dc, :],
                                start=(dc == 0),
                                stop=(dc == DC - 1),
                            )
                    for hc in hcs:
                        if has_bias:
                            nc.vector.tensor_scalar_add(
                                dst[:, hc, ts(tc_i, 512)], ptiles[hc][:], bq_sb[:, hc : hc + 1]
                            )
                        else:
                            nc.vector.tensor_copy(dst[:, hc, ts(tc_i, 512)], ptiles[hc][:])

        if phases == "proj":
            for qt in range(4):
                o_sb = opool.tile([128, 512], F32, tag="o", name="o_probe")
                nc.vector.tensor_copy(o_sb[:], qhT[:, qt, 0:512])
                nc.sync.dma_start(out_d[qt, :, 0:512], o_sb[:])
            return
        if phases in ("sx", "sonly", "sc"):
            nc.vector.memset(qhT[:], 0.01)
            nc.vector.memset(khT[:], 0.01)
            if phases == "sc":
                nc.vector.memset(vha[:], 0.01)
                nc.vector.memset(vha[:, :, :, D_V : D_V + 1], 1.0)

        # ---------------- phase B: streamed attention ----------------
        with (
            tc.tile_pool(name="vstageb", bufs=2) as vstageb,
            tc.tile_pool(name="expool", bufs=EXP_BUFS) as expool,
            tc.tile_pool(name="ps", bufs=PS_BUFS, space="PSUM") as ps,
            tc.tile_pool(name="cp", bufs=4, space="PSUM") as cp,
        ):
            exp_tiles = {}
            dummy_e = None
            if phases in ("noexp", "sc"):
                dummy_e = vpool.tile([128, 1024], BF16, tag="dummy_e")
                nc.vector.memset(dummy_e[:], 0.01)

            def scores_slot(p, qc, sc):
                # one PE slot: both heads of pair p via row-tiled K=64 matmuls
                s_ps = ps.tile([128, 1024], F32, tag="ps", name="s_ps")
                for hl in range(2):
                    pb = hl * 64
                    nc.tensor.matmul(
                        s_ps[:, ts(hl, 512)],
                        lhsT=khT[pb : pb + 64, p, ts(sc, 128)],
                        rhs=qhT[pb : pb + 64, p, ts(qc, 512)],
                        start=True,
                        stop=True,
                    )
                if phases == "sonly":
                    return
                if phases in ("noexp", "sc"):
                    exp_tiles[(p, sc, qc)] = dummy_e
                    return
                e = expool.tile([128, 1024], BF16, tag="exp", name="e_t")
                nc.scalar.activation(
                    e[:], s_ps[:], mybir.ActivationFunctionType.Exp, scale=0.125
                )
                exp_tiles[(p, sc, qc)] = e

            def vh_units():
                # vh: psum (128 s-tile, 512 hd) = xT[d, s].T @ W[d], contraction
                # split into d%128 halves 0-63 / 64-127 — the two K=64 chains
                # run concurrently on the row-tiled PE array (full-array K=128
                # matmuls would serialize against the scores/ctx pair stream).
                # DVE merges the two PSUM partials into vha.
                for vc in range(SCN // 2):
                    xb = vstageb.tile([128, 2, DC, 128], BF16, tag="vxb", name="vxb_t")
                    nc.gpsimd.dma_start(xb[:], xv_d[:, 2 * vc : 2 * vc + 2, :, :])
                    yield
                    for si in range(2):
                        sc = 2 * vc + si
                        pv = cp.tile([128, 512], F32, tag="cp", name="pv_t")
                        for dc in range(DC):
                            nc.tensor.matmul(
                                pv[:],
                                lhsT=xb[:, si, dc, :],
                                rhs=wv_sb[:, dc, :],
                                start=(dc == 0),
                                stop=(dc == DC - 1),
                            )
                            yield
                        nc.vector.tensor_copy(
                            vha[:, sc, :, 0:D_V],
                            pv[:].rearrange("p (h d) -> p h d", d=D_V),
                        )
                        yield

            def ctx_units(p, qc):
                # per head: one K=128 accumulation chain over 16 s-tiles; row
                # 64 = Z (accumulation chains don't row-tile-overlap on HW, so
                # a single full-K chain halves the instruction count).
                for hl in range(2):
                    h = 2 * p + hl
                    pb = hl * 64
                    c = cp.tile([128, 512], F32, tag="cp", name="c_t")
                    for sc in range(SCN):
                        e = exp_tiles[(p, sc, qc)]
                        nc.tensor.matmul(
                            c[0 : D_V + 1, :],
                            lhsT=vha[:, sc, h, :],
                            rhs=e[:, ts(hl, 512)],
                            start=(sc == 0),
                            stop=(sc == SCN - 1),
                        )
                        yield
                    rz = zpool.tile([1, 512], F32, tag="rz", name="rz_t")
                    nc.vector.reciprocal(rz[:], c[D_V : D_V + 1, :])
                    bc_sb = zpool.tile([64, 512], F32, tag="bcs", name="bc_sb")
                    nc.gpsimd.partition_broadcast(bc_sb[:], rz[:], channels=64)
                    nc.vector.tensor_mul(
                        ctxT[pb : pb + 64, p, ts(qc, 512)], c[0:D_V, :], bc_sb[:]
                    )
                    yield
                if phases not in ("noexp", "sc"):
                    for sc in range(SCN):
                        del exp_tiles[(p, sc, qc)]

            def k2_units():
                # background reload of xk to produce khT for hc2/hc3, also as
                # row-tiled K=64 half-chains + DVE merge (see vh_units).
                for kc in range(DC):
                    xb = vstageb.tile([128, DC, 256], BF16, tag="vxb", name="kxb_t")
                    nc.scalar.dma_start(xb[:], xk_d[:, :, ts(kc, 256)])
                    yield
                    for hc in (2, 3):
                        pt = cp.tile([128, 512], F32, tag="cp", name="kp_t")
                        for dc in range(DC):
                            nc.tensor.matmul(
                                pt[0:128, 0:256],
                                lhsT=wk_sb[:, dc, ts(hc, 128)],
                                rhs=xb[:, dc, :],
                                start=(dc == 0),
                                stop=(dc == DC - 1),
                            )
                            yield
                        nc.vector.tensor_copy(khT[:, hc, ts(kc, 256)], pt[0:128, 0:256])
                        yield

            def out_units(qc):
                for qt in range(4 * qc, 4 * qc + 4):
                    po = [
                        cp.tile([128, 512], F32, tag="cp", name="po_t") for _ in range(2)
                    ]
                    for hc in range(HCN):
                        for d2 in range(2):
                            nc.tensor.matmul(
                                po[d2][:],
                                lhsT=ctxT[:, hc, ts(qt, 128)],
                                rhs=wo_sb[:, hc, ts(d2, 512)],
                                start=(hc == 0),
                                stop=(hc == HCN - 1),
                            )
                        yield
                    for d2 in range(2):
                        o_sb = opool.tile([128, 512], F32, tag="o", name="o_sb")
                        nc.vector.tensor_copy(o_sb[:], po[d2][:])
                        nc.sync.dma_start(out_d[qt, :, ts(d2, 512)], o_sb[:])
                    yield

            # urgent: ctx/out work that must land within its step (e-pool
            # recycling depends on it); bg: vh proj + khT hc2/3 reload.
            urgent = deque()
            bg = deque()

            def drain_from(q, n):
                done = 0
                while done < n and q:
                    try:
                        next(q[0])
                        done += 1
                    except StopIteration:
                        q.popleft()

            def drain_all():
                while urgent:
                    drain_from(urgent, 10**9)
                while bg:
                    drain_from(bg, 10**9)

            if phases not in ("sx", "sonly", "sc"):
                bg.append(vh_units())
                bg.append(k2_units())

            # Emission-order invariant (deadlock safety): scores slot allocating
            # exp-buf #N stalls ScalarE until the ctx matmuls consuming buf
            # #(N-EXP_BUFS) run; those must already be EMITTED (earlier in the
            # PE queue). E=32 covers 2 macro-steps of lookahead, so ctx(ms-1)
            # must be fully emitted during step ms — hence vh proj (144 units)
            # must drain entirely within step 0 (budget 9/slot), and ctx+out
            # (<=46 units) within one step (budget 3/slot + 8 boundary).
            # Burst interleaving: keep each PE stretch pattern-homogeneous
            # (scores pairs / ctx pairs / bg pairs) so row-tiled concurrency
            # pipelines instead of restarting at every instruction-mix switch.
            for ms in range(NH // 2 * QCN):
                p, qc = divmod(ms, QCN)
                for burst in range(SCN // 2):
                    scores_slot(p, qc, 2 * burst)
                    scores_slot(p, qc, 2 * burst + 1)
                    if ms == 0:
                        drain_from(bg, 19)
                    else:
                        drain_from(urgent, 6)
                        drain_from(bg, 4)
                drain_from(urgent, 4)
                drain_from(bg, 4)
                if phases == "sonly":
                    continue
                if phases == "sx":
                    for sc in range(SCN):
                        del exp_tiles[(p, sc, qc)]
                    continue
                urgent.append(ctx_units(p, qc))
                if p == NH // 2 - 1 and phases != "sc":
                    urgent.append(out_units(qc))
            drain_all()

            if phases in ("sx", "sonly", "sc"):
                for qt in range(4):
                    o_sb = opool.tile([128, 512], F32, tag="o", name="o_probe2")
                    nc.vector.tensor_copy(o_sb[:], qhT[:, qt, 0:512])
                    nc.sync.dma_start(out_d[qt, :, 0:512], o_sb[:])


# ---------------------------------------------------------------------------
# host side
# ---------------------------------------------------------------------------

_NC_CACHE = {}


def _get_nc(reps: int = 1):
    if reps not in _NC_CACHE:
        _NC_CACHE[reps] = build_nc(reps)
    return _NC_CACHE[reps]


def _to_bf16(a):
    return np.ascontiguousarray(a).astype(ml_dtypes.bfloat16)


def make_in_maps(q, k, v, Wq, bq, Wk, bk, Wv, bv, Wo, bo):
    """Build the per-core input maps (host-side sharding + layout)."""
    in_maps = []
    for c in range(N_CORES):
        b = c // 2
        hg = c % 2
        hs = slice(hg * NH, hg * NH + NH)

        def xt(x):
            # (S, D) -> [p, dc, t] with D = dc*128 + p, shipped as bf16
            return _to_bf16(
                np.asarray(x, np.float32).T.reshape(DC, 128, T).transpose(1, 0, 2)
            )

        def xt_v(x):
            # (S, D) -> [p, sc, dc, t128]: s-tile chunked, bf16
            return _to_bf16(
                np.asarray(x, np.float32).T.reshape(DC, 128, SCN, 128).transpose(1, 2, 0, 3)
            )

        def wproj(W):
            # (8, 1024, 64) -> [p, dc, hd]  (hd = h*64+dv, D = dc*128+p)
            Wc = np.asarray(W[hs], np.float32).transpose(1, 0, 2).reshape(D_MODEL, HD)
            return _to_bf16(Wc.reshape(DC, 128, HD).transpose(1, 0, 2))

        wo_c = np.asarray(Wo[hg * HD : (hg + 1) * HD], np.float32)  # (512, 1024)
        bq_c = np.asarray(bq[hs], np.float32).reshape(HD)  # (512,)

        in_maps.append(
            {
                "xq_t": xt(q[b]),
                "xk_t": xt(k[b]),
                "xv_t": xt_v(v[b]),
                "wq": wproj(Wq),
                "wk": wproj(Wk),
                "wv": wproj(Wv),
                "wo": _to_bf16(wo_c.reshape(HCN, 128, D_MODEL).transpose(1, 0, 2)),
                "bq": np.ascontiguousarray(bq_c.reshape(HCN, 128).T),
            }
        )
    return in_maps


def combine_outputs(results, bv, Wo, bo):
    """results: list of 8 dicts with 'out' (16,128,1024). Returns (B,S,D)."""
    bo_eff = np.asarray(bo, np.float32) + np.asarray(bv, np.float32).reshape(-1) @ np.asarray(
        Wo, np.float32
    )
    out = np.empty((B, S, D_MODEL), np.float32)
    for b in range(B):
        p0 = results[2 * b]["out"].reshape(S, D_MODEL)
        p1 = results[2 * b + 1]["out"].reshape(S, D_MODEL)
        out[b] = p0 + p1 + bo_eff
    return out


def kernel(q, k, v, Wq, bq, Wk, bk, Wv, bv, Wo, bo):
    from concourse.bass_utils import run_bass_kernel_spmd

    nc = _get_nc(1)
    in_maps = make_in_maps(q, k, v, Wq, bq, Wk, bk, Wv, bv, Wo, bo)
    res = run_bass_kernel_spmd(nc, in_maps, core_ids=list(range(N_CORES)))
    return combine_outputs(res.results, bv, Wo, bo)


# revision 50
# speedup vs baseline: 1.0650x; 1.0505x over previous
"""Multi-head attention Trainium2 kernel (v2: interleaved PE/ScalarE stream).

Problem: B=4, S=2048, D_MODEL=1024, H=16 heads, d_k=d_v=64.

Sharding (8 cores, no collectives): core c handles batch b=c//2 and head
group g=c%2 (8 heads). Each core computes its 8 heads' attention and the
partial output projection ctx @ Wo[g's rows]; the host sums the two
head-group partials per batch and adds the (folded) biases.

Math notes:
 - bk drops out of softmax exactly; bv and bo fold into a host-side row
   vector bo_eff = bo + bv@Wo; softmax without max subtraction (|score|<~3).
 - matmuls in bf16 with fp32 PSUM accumulation.

Structure (measured-driven):
 - x shipped bf16 from host (halves HBM traffic; device matmuls were bf16
   anyway, bit-identical). xv pre-chunked per s-tile for contiguous DMA.
 - q/k/v loads spread across the sync/scalar/gpsimd DGE rings.
 - Prelude: q proj fully + k proj pairs 0-1; khT for pairs 2-3 comes from a
   background xk reload inside the stream (shorter dead head before the
   first scores/exp).
 - Stream of macro-steps (pair p, q-chunk qc): 2 scores slots (row-tiled
   K=64 pairs DO run concurrently for single-shot matmuls), then bursts of
   the previous step's ctx chains (urgent) and vh-proj/khT-reload units
   (background). ScalarE does exp ONLY (33.5M elems ~ 266us, the pacing
   engine); all PSUM->SBUF copies are on DVE; 1/Z partition-broadcast on
   GPSIMD.
 - ctx/vh/k2/out use single K=128 accumulation chains: row-tiled K=64
   accumulation-chain pairs do NOT overlap on HW (measured ~190ns/MM vs
   ~98ns/MM for single-shot pairs), so full-K chains halve the instruction
   count for the same streaming time. Z comes free as ctx row 64 (ones
   column appended to vh).
"""

import numpy as np
import ml_dtypes
from collections import deque

import concourse.bass as bass
import concourse.bacc as bacc
import concourse.mybir as mybir
import concourse.tile as tile
from concourse.bass import ts

BF16 = mybir.dt.bfloat16
F32 = mybir.dt.float32

D_MODEL, D_K, D_V, N_HEADS = 1024, 64, 64, 16
B, S = 4, 2048
N_CORES = 8
NH = 8            # heads per core
HD = NH * D_V     # 512, stacked head dim per core
T = S             # tokens per core (one batch)
DC = 8            # D_MODEL / 128 chunks
TCN = 4           # token chunks of 512 for projections
SCN = 16          # s tiles of 128
QCN = 4           # q chunks of 512
HCN = 4           # hd chunks of 128 (2 heads each)
EXP_BUFS = 36
PS_BUFS = 2


def build_nc(reps: int = 1, phases: str = "all"):
    nc = bacc.Bacc("TRN2", target_bir_lowering=False, debug=False)

    xq_d = nc.dram_tensor("xq_t", [128, DC, T], BF16, kind="ExternalInput")
    xk_d = nc.dram_tensor("xk_t", [128, DC, T], BF16, kind="ExternalInput")
    # xv pre-chunked by s-tile on host: [p, sc, dc, t128] -> 4KB DMA lines
    xv_d = nc.dram_tensor("xv_t", [128, SCN, DC, 128], BF16, kind="ExternalInput")
    wq_d = nc.dram_tensor("wq", [128, DC, HD], BF16, kind="ExternalInput")
    wk_d = nc.dram_tensor("wk", [128, DC, HD], BF16, kind="ExternalInput")
    wv_d = nc.dram_tensor("wv", [128, DC, HD], BF16, kind="ExternalInput")
    wo_d = nc.dram_tensor("wo", [128, HCN, D_MODEL], BF16, kind="ExternalInput")
    bq_d = nc.dram_tensor("bq", [128, HCN], F32, kind="ExternalInput")
    out_d = nc.dram_tensor("out", [SCN, 128, D_MODEL], F32, kind="ExternalOutput")

    with tile.TileContext(nc) as tc:
        def body():
            emit_body(nc, tc, xq_d, xk_d, xv_d, wq_d, wk_d, wv_d, wo_d, bq_d, out_d, phases)

        if reps == 1:
            body()
        else:
            with tc.For_i(0, reps, 1):
                body()
    nc.compile()
    return nc


def emit_body(nc, tc, xq_d, xk_d, xv_d, wq_d, wk_d, wv_d, wo_d, bq_d, out_d, phases="all"):
    import contextlib

    ctx = contextlib.ExitStack()
    with ctx:
        # ---------------- persistent SBUF pools ----------------
        wpool = ctx.enter_context(tc.tile_pool(name="wpool", bufs=1))
        qkpool = ctx.enter_context(tc.tile_pool(name="qkpool", bufs=1))
        vpool = ctx.enter_context(tc.tile_pool(name="vpool", bufs=1))
        cpool = ctx.enter_context(tc.tile_pool(name="cpool", bufs=1))
        opool = ctx.enter_context(tc.tile_pool(name="opool", bufs=4))
        zpool = ctx.enter_context(tc.tile_pool(name="zpool", bufs=2))

        wq_sb = wpool.tile([128, DC, HD], BF16, tag="wq")
        wk_sb = wpool.tile([128, DC, HD], BF16, tag="wk")
        wv_sb = wpool.tile([128, DC, HD], BF16, tag="wv")
        wo_sb = wpool.tile([128, HCN, D_MODEL], BF16, tag="wo")
        bq_sb = wpool.tile([128, HCN], F32, tag="bq")

        # weights on the gpsimd DGE ring: parallel with x loads on sync
        nc.gpsimd.dma_start(wq_sb[:], wq_d[:])
        nc.gpsimd.dma_start(bq_sb[:], bq_d[:])
        nc.gpsimd.dma_start(wk_sb[:], wk_d[:])
        nc.gpsimd.dma_start(wv_sb[:], wv_d[:])
        nc.gpsimd.dma_start(wo_sb[:], wo_d[:])

        qhT = qkpool.tile([128, HCN, T], BF16, tag="qhT")  # [hd%128, hd//128, t]
        khT = qkpool.tile([128, HCN, T], BF16, tag="khT")
        vha = vpool.tile([128, SCN, NH, D_V + 1], BF16, tag="vha")  # [s%128, s//128, h, dv|1]
        ctxT = cpool.tile([128, HCN, T], BF16, tag="ctxT")

        nc.vector.memset(vha[:, :, :, D_V : D_V + 1], 1.0)

        if phases == "min":
            # near-empty body: measures per-iteration fixed overhead
            for qt in range(4):
                o_sb = opool.tile([128, 512], F32, tag="o", name="o_min")
                nc.vector.memset(o_sb[:], 0.0)
                nc.sync.dma_start(out_d[qt, :, 0:512], o_sb[:])
            return

        if phases == "dmaonly":
            # all input DMA traffic, no compute
            nc.gpsimd.dma_start(wq_sb[:], wq_d[:])
            nc.gpsimd.dma_start(wk_sb[:], wk_d[:])
            nc.gpsimd.dma_start(wv_sb[:], wv_d[:])
            nc.gpsimd.dma_start(wo_sb[:], wo_d[:])
            with tc.tile_pool(name="dstage", bufs=3) as dstage:
                for tc_i in range(TCN):
                    xbq = dstage.tile([128, DC, 512], BF16, tag="dq", name="dq_t")
                    nc.sync.dma_start(xbq[:], xq_d[:, :, ts(tc_i, 512)])
                    xbk = dstage.tile([128, DC, 512], BF16, tag="dk", name="dk_t")
                    nc.scalar.dma_start(xbk[:], xk_d[:, :, ts(tc_i, 512)])
                for vc in range(SCN // 2):
                    xbv = dstage.tile([128, 2, DC, 128], BF16, tag="dv", name="dv_t")
                    nc.gpsimd.dma_start(xbv[:], xv_d[:, 2 * vc : 2 * vc + 2, :, :])
            for qt in range(4):
                o_sb = opool.tile([128, 512], F32, tag="o", name="o_dma")
                nc.vector.memset(o_sb[:], 0.0)
                nc.sync.dma_start(out_d[qt, :, 0:512], o_sb[:])
            return

        # ---------------- phase A: q/k projections ----------------
        if phases in ("all", "proj", "noexp"):
         with (
            tc.tile_pool(name="xbf", bufs=3) as xbfp,
            tc.tile_pool(name="pp", bufs=6, space="PSUM") as pp,
        ):
            # chunk-interleaved q/k: q chunks load on the sync DGE ring, k
            # chunks on the scalar ring (ScalarE is idle until the first
            # scores land) — the two 8MB streams overlap instead of
            # serializing, and MMs pipeline behind both.
            # Prelude: q fully; k only head-chunks hc0/hc1 (pairs 0-1). khT for
            # hc2/hc3 is produced by a background reload pass inside the
            # stream (k2_units) — shortens the dead head before first scores.
            for tc_i in range(TCN):
                for x_d, w_sb, dst, has_bias, hcs in (
                    (xq_d, wq_sb, qhT, True, range(HCN)),
                    (xk_d, wk_sb, khT, False, range(2)),
                ):
                    xb = xbfp.tile([128, DC, 512], BF16, tag="xb", name="xb_t")
                    (nc.sync if has_bias else nc.scalar).dma_start(
                        xb[:], x_d[:, :, ts(tc_i, 512)]
                    )
                    ptiles = {hc: pp.tile([128, 512], F32, tag="pp", name="pp_t") for hc in hcs}
                    for dc in range(DC):
                        for hc in hcs:
                            nc.tensor.matmul(
                                ptiles[hc][:],
                                lhsT=w_sb[:, dc, ts(hc, 128)],
                                rhs=xb[:, dc, :],
                                start=(dc == 0),
                                stop=(dc == DC - 1),
                            )
                    for hc in hcs:
                        if has_bias:
                            nc.vector.tensor_scalar_add(
                                dst[:, hc, ts(tc_i, 512)], ptiles[hc][:], bq_sb[:, hc : hc + 1]
                            )
                        else:
                            nc.vector.tensor_copy(dst[:, hc, ts(tc_i, 512)], ptiles[hc][:])

        if phases == "proj":
            for qt in range(4):
                o_sb = opool.tile([128, 512], F32, tag="o", name="o_probe")
                nc.vector.tensor_copy(o_sb[:], qhT[:, qt, 0:512])
                nc.sync.dma_start(out_d[qt, :, 0:512], o_sb[:])
            return
        if phases in ("sx", "sonly", "sc"):
            nc.vector.memset(qhT[:], 0.01)
            nc.vector.memset(khT[:], 0.01)
            if phases == "sc":
                nc.vector.memset(vha[:], 0.01)
                nc.vector.memset(vha[:, :, :, D_V : D_V + 1], 1.0)

        # ---------------- phase B: streamed attention ----------------
        with (
            tc.tile_pool(name="vstageb", bufs=2) as vstageb,
            tc.tile_pool(name="expool", bufs=EXP_BUFS) as expool,
            tc.tile_pool(name="ps", bufs=PS_BUFS, space="PSUM") as ps,
            tc.tile_pool(name="cp", bufs=4, space="PSUM") as cp,
        ):
            exp_tiles = {}
            dummy_e = None
            if phases in ("noexp", "sc"):
                dummy_e = vpool.tile([128, 1024], BF16, tag="dummy_e")
                nc.vector.memset(dummy_e[:], 0.01)

            def scores_slot(p, qc, sc):
                # one PE slot: both heads of pair p via row-tiled K=64 matmuls
                s_ps = ps.tile([128, 1024], F32, tag="ps", name="s_ps")
                for hl in range(2):
                    pb = hl * 64
                    nc.tensor.matmul(
                        s_ps[:, ts(hl, 512)],
                        lhsT=khT[pb : pb + 64, p, ts(sc, 128)],
                        rhs=qhT[pb : pb + 64, p, ts(qc, 512)],
                        start=True,
                        stop=True,
                    )
                if phases == "sonly":
                    return
                if phases in ("noexp", "sc"):
                    exp_tiles[(p, sc, qc)] = dummy_e
                    return
                e = expool.tile([128, 1024], BF16, tag="exp", name="e_t")
                nc.scalar.activation(
                    e[:], s_ps[:], mybir.ActivationFunctionType.Exp, scale=0.125
                )
                exp_tiles[(p, sc, qc)] = e

            def vh_units():
                # vh: psum (128 s-tile, 512 hd) = xT[d, s].T @ W[d], contraction
                # split into d%128 halves 0-63 / 64-127 — the two K=64 chains
                # run concurrently on the row-tiled PE array (full-array K=128
                # matmuls would serialize against the scores/ctx pair stream).
                # DVE merges the two PSUM partials into vha.
                for vc in range(SCN // 2):
                    xb = vstageb.tile([128, 2, DC, 128], BF16, tag="vxb", name="vxb_t")
                    nc.gpsimd.dma_start(xb[:], xv_d[:, 2 * vc : 2 * vc + 2, :, :])
                    yield
                    for si in range(2):
                        sc = 2 * vc + si
                        pv = cp.tile([128, 512], F32, tag="cp", name="pv_t")
                        for dc in range(DC):
                            nc.tensor.matmul(
                                pv[:],
                                lhsT=xb[:, si, dc, :],
                                rhs=wv_sb[:, dc, :],
                                start=(dc == 0),
                                stop=(dc == DC - 1),
                            )
                            yield
                        nc.vector.tensor_copy(
                            vha[:, sc, :, 0:D_V],
                            pv[:].rearrange("p (h d) -> p h d", d=D_V),
                        )
                        yield

            def ctx_units(p, qc):
                # per head: one K=128 accumulation chain over 16 s-tiles; row
                # 64 = Z (accumulation chains don't row-tile-overlap on HW, so
                # a single full-K chain halves the instruction count).
                for hl in range(2):
                    h = 2 * p + hl
                    pb = hl * 64
                    c = cp.tile([128, 512], F32, tag="cp", name="c_t")
                    for sc in range(SCN):
                        e = exp_tiles[(p, sc, qc)]
                        nc.tensor.matmul(
                            c[0 : D_V + 1, :],
                            lhsT=vha[:, sc, h, :],
                            rhs=e[:, ts(hl, 512)],
                            start=(sc == 0),
                            stop=(sc == SCN - 1),
                        )
                        yield
                    rz = zpool.tile([1, 512], F32, tag="rz", name="rz_t")
                    nc.vector.reciprocal(rz[:], c[D_V : D_V + 1, :])
                    bc_sb = zpool.tile([64, 512], F32, tag="bcs", name="bc_sb")
                    nc.gpsimd.partition_broadcast(bc_sb[:], rz[:], channels=64)
                    nc.vector.tensor_mul(
                        ctxT[pb : pb + 64, p, ts(qc, 512)], c[0:D_V, :], bc_sb[:]
                    )
                    yield
                if phases not in ("noexp", "sc"):
                    for sc in range(SCN):
                        del exp_tiles[(p, sc, qc)]

            def k2_units():
                # background reload of xk to produce khT for hc2/hc3, also as
                # row-tiled K=64 half-chains + DVE merge (see vh_units).
                for kc in range(DC):
                    xb = vstageb.tile([128, DC, 256], BF16, tag="vxb", name="kxb_t")
                    nc.scalar.dma_start(xb[:], xk_d[:, :, ts(kc, 256)])
                    yield
                    for hc in (2, 3):
                        pt = cp.tile([128, 512], F32, tag="cp", name="kp_t")
                        for dc in range(DC):
                            nc.tensor.matmul(
                                pt[0:128, 0:256],
                                lhsT=wk_sb[:, dc, ts(hc, 128)],
                                rhs=xb[:, dc, :],
                                start=(dc == 0),
                                stop=(dc == DC - 1),
                            )
                            yield
                        nc.vector.tensor_copy(khT[:, hc, ts(kc, 256)], pt[0:128, 0:256])
                        yield

            def out_units(qc):
                for qt in range(4 * qc, 4 * qc + 4):
                    po = [
                        cp.tile([128, 512], F32, tag="cp", name="po_t") for _ in range(2)
                    ]
                    for hc in range(HCN):
                        for d2 in range(2):
                            nc.tensor.matmul(
                                po[d2][:],
                                lhsT=ctxT[:, hc, ts(qt, 128)],
                                rhs=wo_sb[:, hc, ts(d2, 512)],
                                start=(hc == 0),
                                stop=(hc == HCN - 1),
                            )
                        yield
                    for d2 in range(2):
                        o_sb = opool.tile([128, 512], F32, tag="o", name="o_sb")
                        nc.vector.tensor_copy(o_sb[:], po[d2][:])
                        nc.sync.dma_start(out_d[qt, :, ts(d2, 512)], o_sb[:])
                    yield

            # urgent: ctx/out work that must land within its step (e-pool
            # recycling depends on it); bg: vh proj + khT hc2/3 reload.
            urgent = deque()
            bg = deque()

            def drain_from(q, n):
                done = 0
                while done < n and q:
                    try:
                        next(q[0])
                        done += 1
                    except StopIteration:
                        q.popleft()

            def drain_all():
                while urgent:
                    drain_from(urgent, 10**9)
                while bg:
                    drain_from(bg, 10**9)

            if phases not in ("sx", "sonly", "sc"):
                bg.append(vh_units())
                bg.append(k2_units())

            # Emission-order invariant (deadlock safety): scores slot allocating
            # exp-buf #N stalls ScalarE until the ctx matmuls consuming buf
            # #(N-EXP_BUFS) run; those must already be EMITTED (earlier in the
            # PE queue). E=32 covers 2 macro-steps of lookahead, so ctx(ms-1)
            # must be fully emitted during step ms — hence vh proj (144 units)
            # must drain entirely within step 0 (budget 9/slot), and ctx+out
            # (<=46 units) within one step (budget 3/slot + 8 boundary).
            # Fine per-slot interleaving: spreads the PE's exp-bound idle into
            # many sub-us gaps (chunky burst schedules measured worse —
            # consistent with HAM re-throttling on longer PE-idle gaps).
            for ms in range(NH // 2 * QCN):
                p, qc = divmod(ms, QCN)
                for sc in range(SCN):
                    if ms == 0:
                        drain_from(bg, 10)
                    else:
                        drain_from(urgent, 3)
                        drain_from(bg, 2)
                    scores_slot(p, qc, sc)
                drain_from(urgent, 4)
                drain_from(bg, 4)
                if phases == "sonly":
                    continue
                if phases == "sx":
                    for sc in range(SCN):
                        del exp_tiles[(p, sc, qc)]
                    continue
                urgent.append(ctx_units(p, qc))
                if p == NH // 2 - 1 and phases != "sc":
                    urgent.append(out_units(qc))
            drain_all()

            if phases in ("sx", "sonly", "sc"):
                for qt in range(4):
                    o_sb = opool.tile([128, 512], F32, tag="o", name="o_probe2")
                    nc.vector.tensor_copy(o_sb[:], qhT[:, qt, 0:512])
                    nc.sync.dma_start(out_d[qt, :, 0:512], o_sb[:])


# ---------------------------------------------------------------------------
# host side
# ---------------------------------------------------------------------------

_NC_CACHE = {}


def _get_nc(reps: int = 1):
    if reps not in _NC_CACHE:
        _NC_CACHE[reps] = build_nc(reps)
    return _NC_CACHE[reps]


def _to_bf16(a):
    return np.ascontiguousarray(a).astype(ml_dtypes.bfloat16)


def make_in_maps(q, k, v, Wq, bq, Wk, bk, Wv, bv, Wo, bo):
    """Build the per-core input maps (host-side sharding + layout)."""
    in_maps = []
    for c in range(N_CORES):
        b = c // 2
        hg = c % 2
        hs = slice(hg * NH, hg * NH + NH)

        def xt(x):
            # (S, D) -> [p, dc, t] with D = dc*128 + p, shipped as bf16
            return _to_bf16(
                np.asarray(x, np.float32).T.reshape(DC, 128, T).transpose(1, 0, 2)
            )

        def xt_v(x):
            # (S, D) -> [p, sc, dc, t128]: s-tile chunked, bf16
            return _to_bf16(
                np.asarray(x, np.float32).T.reshape(DC, 128, SCN, 128).transpose(1, 2, 0, 3)
            )

        def wproj(W):
            # (8, 1024, 64) -> [p, dc, hd]  (hd = h*64+dv, D = dc*128+p)
            Wc = np.asarray(W[hs], np.float32).transpose(1, 0, 2).reshape(D_MODEL, HD)
            return _to_bf16(Wc.reshape(DC, 128, HD).transpose(1, 0, 2))

        wo_c = np.asarray(Wo[hg * HD : (hg + 1) * HD], np.float32)  # (512, 1024)
        bq_c = np.asarray(bq[hs], np.float32).reshape(HD)  # (512,)

        in_maps.append(
            {
                "xq_t": xt(q[b]),
                "xk_t": xt(k[b]),
                "xv_t": xt_v(v[b]),
                "wq": wproj(Wq),
                "wk": wproj(Wk),
                "wv": wproj(Wv),
                "wo": _to_bf16(wo_c.reshape(HCN, 128, D_MODEL).transpose(1, 0, 2)),
                "bq": np.ascontiguousarray(bq_c.reshape(HCN, 128).T),
            }
        )
    return in_maps


def combine_outputs(results, bv, Wo, bo):
    """results: list of 8 dicts with 'out' (16,128,1024). Returns (B,S,D)."""
    bo_eff = np.asarray(bo, np.float32) + np.asarray(bv, np.float32).reshape(-1) @ np.asarray(
        Wo, np.float32
    )
    out = np.empty((B, S, D_MODEL), np.float32)
    for b in range(B):
        p0 = results[2 * b]["out"].reshape(S, D_MODEL)
        p1 = results[2 * b + 1]["out"].reshape(S, D_MODEL)
        out[b] = p0 + p1 + bo_eff
    return out


def kernel(q, k, v, Wq, bq, Wk, bk, Wv, bv, Wo, bo):
    from concourse.bass_utils import run_bass_kernel_spmd

    nc = _get_nc(1)
    in_maps = make_in_maps(q, k, v, Wq, bq, Wk, bk, Wv, bv, Wo, bo)
    res = run_bass_kernel_spmd(nc, in_maps, core_ids=list(range(N_CORES)))
    return combine_outputs(res.results, bv, Wo, bo)
